# revision 6
# baseline (speedup 1.0000x reference)
"""LocalWindowAttention (block-causal) Trainium2 kernel, 8 NeuronCores.

Sharding: tensor-parallel over heads. Core c owns head-columns
[c*128, (c+1)*128) of the D=1024 hidden dim (2 heads x head_dim 64):
  - computes Q/K/V projections for its head slice (transposed layout),
  - block-causal attention for its 2 heads,
  - partial output projection with its 128 rows of Wo,
  - ReduceScatter(add) sums partials; core c keeps rows [c*128,(c+1)*128)
    of final^T [1024, 2048]. Host concatenates + transposes.

All big matmuls run in float32r (fp32 with ~13-bit mantissa rounding on
the PE read path) which streams at 1 cycle/row for free dim >= 256 --
4x faster than plain fp32, ~32x more precise than bf16.

Attention is computed in S^T layout (keys on partitions, queries on the
free dim): S^T tile = K_chunk @ Q^T. Softmax needs no max-subtraction
(scores are bounded ~|s|<4 here), and the row-sum comes free by
appending a ones-column to the V operand of the attn@V matmul
(out row 64 = sum_k exp(s)). Normalization divides at the end.
"""

import numpy as np

import concourse.bacc as bacc
import concourse.tile as tile
from concourse import mybir
from concourse.bass_utils import run_bass_kernel_spmd
from concourse.masks import make_identity

B, T, D = 1, 2048, 1024
H, HD, W = 16, 64, 128
N_CORES = 8
HS = D // N_CORES        # 128 head-columns per core (2 heads)
HPC = H // N_CORES       # heads per core
QW = 512                 # query-chunk width (free dim of S^T tiles)
NQ = T // QW             # 4 query chunks
NK = T // W              # 16 key chunks of 128
ND = D // 128            # 8 contraction chunks over D
SCALE = HD ** -0.5

F32 = mybir.dt.float32
F32R = mybir.dt.float32r
Exp = mybir.ActivationFunctionType.Exp

_compiled = {}


def _build():
    nc = bacc.Bacc("TRN2", target_bir_lowering=False, debug=False,
                   num_devices=N_CORES)
    xT_ap = nc.dram_tensor("xT", [D, T], F32R, kind="ExternalInput").ap()
    wq_ap = nc.dram_tensor("wq", [D, HS], F32R, kind="ExternalInput").ap()
    wk_ap = nc.dram_tensor("wk", [D, HS], F32R, kind="ExternalInput").ap()
    wv_ap = nc.dram_tensor("wv", [D, HS], F32R, kind="ExternalInput").ap()
    wo_ap = nc.dram_tensor("wo", [HS, D], F32R, kind="ExternalInput").ap()
    y_ap = nc.dram_tensor("y", [HS, T], F32, kind="ExternalOutput").ap()

    with tile.TileContext(nc) as tc:
        _body(tc, xT_ap, wq_ap, wk_ap, wv_ap, wo_ap, y_ap)
    nc.compile()
    return nc


def _body(tc, xT_ap, wq_ap, wk_ap, wv_ap, wo_ap, y_ap):
    nc = tc.nc
    from contextlib import ExitStack
    with ExitStack() as ctx:
        singles = ctx.enter_context(tc.tile_pool(name="singles", bufs=1))
        work = ctx.enter_context(tc.tile_pool(name="work", bufs=4))
        dram = ctx.enter_context(tc.tile_pool(name="dram", bufs=1, space="DRAM"))

        # ---- load inputs --------------------------------------------------
        xts = []
        for d in range(ND):
            xt = singles.tile([128, T], F32R, tag=f"x{d}")
            nc.sync.dma_start(out=xt[:], in_=xT_ap[d * 128:(d + 1) * 128, :])
            xts.append(xt)
        wq = singles.tile([128, ND, HS], F32R, tag="wq")
        wk = singles.tile([128, ND, HS], F32R, tag="wk")
        wv = singles.tile([128, ND, HS], F32R, tag="wv")
        nc.sync.dma_start(out=wq[:], in_=wq_ap.rearrange("(c p) m -> p c m", p=128))
        nc.sync.dma_start(out=wk[:], in_=wk_ap.rearrange("(c p) m -> p c m", p=128))
        nc.sync.dma_start(out=wv[:], in_=wv_ap.rearrange("(c p) m -> p c m", p=128))
        wo = singles.tile([128, ND, 128], F32R, tag="wo")
        nc.sync.dma_start(out=wo[:], in_=wo_ap.rearrange("p (o n) -> p o n", o=ND))

        ident_f32 = singles.tile([128, 128], F32, tag="ident_f32")
        make_identity(nc, ident_f32)
        ident = singles.tile([128, 128], F32R, tag="ident")
        nc.vector.tensor_copy(ident[:], ident_f32[:])

        qT = singles.tile([128, T], F32R, tag="qT")
        kT = singles.tile([128, T], F32R, tag="kT")
        vT = singles.tile([128, T], F32R, tag="vT")
        # V in natural layout per head: [key 128, NK chunks, HD + ones col]
        vn = [singles.tile([128, NK, HD + 1], F32R, tag=f"vn{h}", name=f"vn{h}")
              for h in range(HPC)]
        outT = singles.tile([128, T], F32R, tag="outT")

        # ---- Q/K/V projections (transposed layout) ------------------------
        with tc.tile_pool(name="pp", bufs=2, space="PSUM") as pp:
            for t in range(NQ):
                ps_q = pp.tile([128, QW], F32, tag="q")
                ps_k = pp.tile([128, QW], F32, tag="k")
                ps_v = pp.tile([128, QW], F32, tag="v")
                cols = slice(t * QW, (t + 1) * QW)
                for d in range(ND):
                    f = (d == 0)
                    l = (d == ND - 1)
                    nc.tensor.matmul(ps_q[:], wq[:, d, :], xts[d][:, cols], start=f, stop=l)
                    nc.tensor.matmul(ps_k[:], wk[:, d, :], xts[d][:, cols], start=f, stop=l)
                    nc.tensor.matmul(ps_v[:], wv[:, d, :], xts[d][:, cols], start=f, stop=l)
                nc.vector.tensor_copy(qT[:, cols], ps_q[:])
                nc.vector.tensor_copy(kT[:, cols], ps_k[:])
                nc.vector.tensor_copy(vT[:, cols], ps_v[:])

        # ---- transpose V to natural layout, append ones column ------------
        ones = singles.tile([128, 1], F32, tag="ones")
        nc.vector.memset(ones[:], 1.0)
        for h in range(HPC):
            nc.vector.tensor_copy(vn[h][:, :, HD:], ones[:].unsqueeze(1).to_broadcast([128, NK, 1]))
        with tc.tile_pool(name="pt", bufs=3, space="PSUM") as pt:
            for tk in range(NK):
                ps_t = pt.tile([128, 128], F32R, tag="t")
                nc.tensor.transpose(
                    ps_t[:], vT[:, tk * W:(tk + 1) * W], ident[:])
                for h in range(HPC):
                    nc.vector.tensor_copy(vn[h][:, tk, :HD],
                                          ps_t[:, h * HD:(h + 1) * HD])

        # ---- attention + output projection --------------------------------
        rs_in = dram.tile([D, T], F32)

        with tc.tile_pool(name="pa", bufs=3, space="PSUM") as pa, \
             tc.tile_pool(name="po", bufs=2, space="PSUM") as po, \
             tc.tile_pool(name="pf", bufs=2, space="PSUM") as pf:
            for t in range(NQ):
                cols = slice(t * QW, (t + 1) * QW)
                for h in range(HPC):
                    hrows = slice(h * HD, (h + 1) * HD)
                    ps_o = po.tile([HD + 1, QW], F32, tag="o")
                    n_tk = 4 * t + 4
                    for tk in range(n_tk):
                        qs = max(0, (tk - 4 * t) * W)  # masked cols before qs
                        ps_s = pa.tile([128, QW], F32, tag="s")
                        nc.tensor.matmul(
                            ps_s[:, qs:], kT[hrows, tk * W:(tk + 1) * W],
                            qT[hrows, t * QW + qs:(t + 1) * QW],
                            start=True, stop=True)
                        es = work.tile([128, QW], F32R, tag="es")
                        nc.scalar.activation(out=es[:, qs:], in_=ps_s[:, qs:],
                                             func=Exp, scale=SCALE)
                        nc.tensor.matmul(ps_o[:, qs:], vn[h][:, tk, :], es[:, qs:],
                                         start=(tk == 0), stop=(tk == n_tk - 1))
                    # normalize: rows 0..63 / row 64
                    rec = work.tile([1, QW], F32, tag="rec")
                    nc.vector.reciprocal(out=rec[:], in_=ps_o[HD:, :])
                    bc = work.tile([HD, QW], F32, tag="bc")
                    nc.gpsimd.partition_broadcast(bc[:], rec[:])
                    nc.vector.tensor_mul(outT[hrows, cols], ps_o[:HD, :], bc[:])

                # partial output projection for this query chunk
                for oc in range(ND):
                    ps_f = pf.tile([128, QW], F32, tag="f")
                    nc.tensor.matmul(ps_f[:], wo[:, oc, :], outT[:, cols],
                                     start=True, stop=True)
                    cf = work.tile([128, QW], F32, tag="cf")
                    nc.vector.tensor_copy(cf[:], ps_f[:])
                    nc.sync.dma_start(
                        out=rs_in[oc * 128:(oc + 1) * 128, cols], in_=cf[:])

        # ---- ReduceScatter over the 8 cores ------------------------------
        rs_out = dram.tile([HS, T], F32)
        nc.gpsimd.collective_compute(
            "ReduceScatter", mybir.AluOpType.add,
            replica_groups=[list(range(N_CORES))],
            ins=[rs_in.opt()], outs=[rs_out.opt()])
        nc.sync.dma_start(out=y_ap[:], in_=rs_out[:])


def kernel(x, Wq, Wk, Wv, Wo):
    if "nc" not in _compiled:
        _compiled["nc"] = _build()
    nc = _compiled["nc"]

    xT = np.ascontiguousarray(x.reshape(T, D).T.astype(np.float32))
    in_maps = []
    for c in range(N_CORES):
        hs = slice(c * HS, (c + 1) * HS)
        in_maps.append({
            "xT": xT,
            "wq": np.ascontiguousarray(Wq[:, hs].astype(np.float32)),
            "wk": np.ascontiguousarray(Wk[:, hs].astype(np.float32)),
            "wv": np.ascontiguousarray(Wv[:, hs].astype(np.float32)),
            "wo": np.ascontiguousarray(Wo[hs, :].astype(np.float32)),
        })
    res = run_bass_kernel_spmd(nc, in_maps, list(range(N_CORES)))
    finalT = np.concatenate([res.results[c]["y"] for c in range(N_CORES)], axis=0)
    return np.ascontiguousarray(finalT.T).reshape(B, T, D)


# revision 9
# speedup vs baseline: 1.0778x; 1.0778x over previous
"""LocalWindowAttention (block-causal) Trainium2 kernel, 8 NeuronCores.

Sharding: tensor-parallel over heads. Core c owns head-columns
[c*128, (c+1)*128) of the D=1024 hidden dim (2 heads x head_dim 64):
  - computes Q/K/V projections for its head slice (transposed layout),
  - block-causal attention for its 2 heads,
  - partial output projection with its 128 rows of Wo,
  - chunked ReduceScatter(add) sums partials; core c keeps rows
    [c*128,(c+1)*128) of final^T [1024, 2048]. Host reassembles.

All big matmuls run in float32r (fp32 with ~13-bit mantissa rounding on
the PE read path): 1 cycle/row for free dim >= 256 -- 4x faster than
plain fp32, ~32x more precise than bf16.

Attention runs in S^T layout (keys on partitions, queries free):
S^T tile = K_chunk @ Q^T. No max-subtraction needed (scores bounded),
and the softmax denominator comes free from a ones-column appended to
the V operand of the attn@V matmul (output row 64 = sum_k exp(s)).
The two heads are interleaved so the exp (ACT engine) of one head
hides behind the other head's matmuls, keeping the PE dense and the
HAM clock un-throttled. Query chunks are processed in descending
visibility order so each chunk's partial output projection and its
ReduceScatter slice overlap the remaining attention compute.
"""

import numpy as np

import concourse.bacc as bacc
import concourse.tile as tile
from concourse import mybir
from concourse.bass_utils import run_bass_kernel_spmd
from concourse.masks import make_identity

B, T, D = 1, 2048, 1024
H, HD, W = 16, 64, 128
N_CORES = 8
HS = D // N_CORES        # 128 head-columns per core (2 heads)
HPC = H // N_CORES       # heads per core
QW = 512                 # query-chunk width (free dim of S^T tiles)
NQ = T // QW             # 4 query chunks
NK = T // W              # 16 key chunks of 128
ND = D // 128            # 8 contraction chunks over D
SCALE = HD ** -0.5

F32 = mybir.dt.float32
F32R = mybir.dt.float32r
Exp = mybir.ActivationFunctionType.Exp

_compiled = {}


def _build():
    nc = bacc.Bacc("TRN2", target_bir_lowering=False, debug=False,
                   num_devices=N_CORES)
    xT_ap = nc.dram_tensor("xT", [D, T], F32R, kind="ExternalInput").ap()
    wq_ap = nc.dram_tensor("wq", [D, HS], F32R, kind="ExternalInput").ap()
    wk_ap = nc.dram_tensor("wk", [D, HS], F32R, kind="ExternalInput").ap()
    wv_ap = nc.dram_tensor("wv", [D, HS], F32R, kind="ExternalInput").ap()
    wo_ap = nc.dram_tensor("wo", [HS, D], F32R, kind="ExternalInput").ap()
    y_ap = nc.dram_tensor("y", [HS, T], F32, kind="ExternalOutput").ap()

    with tile.TileContext(nc) as tc:
        _body(tc, xT_ap, wq_ap, wk_ap, wv_ap, wo_ap, y_ap)
    nc.compile()
    return nc


def _body(tc, xT_ap, wq_ap, wk_ap, wv_ap, wo_ap, y_ap):
    nc = tc.nc
    from contextlib import ExitStack
    with ExitStack() as ctx:
        singles = ctx.enter_context(tc.tile_pool(name="singles", bufs=1))
        work = ctx.enter_context(tc.tile_pool(name="work", bufs=4))
        es_pool = ctx.enter_context(tc.tile_pool(name="es_pool", bufs=6))
        dram = ctx.enter_context(tc.tile_pool(name="dram", bufs=1, space="DRAM"))

        # ---- load inputs (weights first: small, unblock first matmuls) ----
        wq = singles.tile([128, ND, HS], F32R, tag="wq")
        wk = singles.tile([128, ND, HS], F32R, tag="wk")
        wv = singles.tile([128, ND, HS], F32R, tag="wv")
        nc.sync.dma_start(out=wq[:], in_=wq_ap.rearrange("(c p) m -> p c m", p=128))
        nc.sync.dma_start(out=wk[:], in_=wk_ap.rearrange("(c p) m -> p c m", p=128))
        nc.sync.dma_start(out=wv[:], in_=wv_ap.rearrange("(c p) m -> p c m", p=128))
        wo = singles.tile([128, ND, 128], F32R, tag="wo")
        nc.sync.dma_start(out=wo[:], in_=wo_ap.rearrange("p (o n) -> p o n", o=ND))
        xts = []
        for d in range(ND):
            xt = singles.tile([128, T], F32R, tag=f"x{d}", name=f"xt{d}")
            nc.sync.dma_start(out=xt[:], in_=xT_ap[d * 128:(d + 1) * 128, :])
            xts.append(xt)

        ident_f32 = singles.tile([128, 128], F32, tag="ident_f32")
        make_identity(nc, ident_f32)
        ident = singles.tile([128, 128], F32R, tag="ident")
        nc.vector.tensor_copy(ident[:], ident_f32[:])

        qT = singles.tile([128, T], F32R, tag="qT")
        kT = singles.tile([128, T], F32R, tag="kT")
        vT = singles.tile([128, T], F32R, tag="vT")
        # V in natural layout per head: [key 128, NK chunks, HD + ones col]
        vn = [singles.tile([128, NK, HD + 1], F32R, tag=f"vn{h}", name=f"vn{h}")
              for h in range(HPC)]
        outT = singles.tile([128, T], F32R, tag="outT")

        # ---- Q/K/V projections (transposed layout) ------------------------
        with tc.tile_pool(name="pp", bufs=2, space="PSUM") as pp:
            for t in range(NQ):
                ps_q = pp.tile([128, QW], F32, tag="q")
                ps_k = pp.tile([128, QW], F32, tag="k")
                ps_v = pp.tile([128, QW], F32, tag="v")
                cols = slice(t * QW, (t + 1) * QW)
                for d in range(ND):
                    f = (d == 0)
                    l = (d == ND - 1)
                    nc.tensor.matmul(ps_q[:], wq[:, d, :], xts[d][:, cols], start=f, stop=l)
                    nc.tensor.matmul(ps_k[:], wk[:, d, :], xts[d][:, cols], start=f, stop=l)
                    nc.tensor.matmul(ps_v[:], wv[:, d, :], xts[d][:, cols], start=f, stop=l)
                nc.vector.tensor_copy(qT[:, cols], ps_q[:])
                nc.vector.tensor_copy(kT[:, cols], ps_k[:])
                nc.vector.tensor_copy(vT[:, cols], ps_v[:])

        # ---- transpose V to natural layout, append ones column ------------
        ones = singles.tile([128, 1], F32, tag="ones")
        nc.vector.memset(ones[:], 1.0)
        for h in range(HPC):
            nc.vector.tensor_copy(vn[h][:, :, HD:],
                                  ones[:].unsqueeze(1).to_broadcast([128, NK, 1]))
        with tc.tile_pool(name="pt", bufs=3, space="PSUM") as pt:
            for tk in range(NK):
                ps_t = pt.tile([128, 128], F32R, tag="t")
                nc.tensor.transpose(
                    ps_t[:], vT[:, tk * W:(tk + 1) * W], ident[:])
                for h in range(HPC):
                    nc.vector.tensor_copy(vn[h][:, tk, :HD],
                                          ps_t[:, h * HD:(h + 1) * HD])

        # ---- attention + output projection, query chunks descending -------
        rs_in = dram.tile([NQ, D, QW], F32)
        rs_out = dram.tile([NQ, HS, QW], F32)

        with tc.tile_pool(name="pa", bufs=2, space="PSUM") as pa, \
             tc.tile_pool(name="po", bufs=2, space="PSUM") as po, \
             tc.tile_pool(name="pf", bufs=2, space="PSUM") as pf:
            for t in range(NQ - 1, -1, -1):
                cols = slice(t * QW, (t + 1) * QW)
                n_tk = 4 * t + 4
                ps_o = [po.tile([HD + 1, QW], F32, tag=f"o{h}", name=f"ps_o{h}")
                        for h in range(HPC)]
                for tk in range(n_tk):
                    qs = max(0, (tk - 4 * t) * W)  # masked cols before qs
                    es = []
                    for h in range(HPC):
                        hrows = slice(h * HD, (h + 1) * HD)
                        ps_s = pa.tile([128, QW], F32, tag="s")
                        nc.tensor.matmul(
                            ps_s[:, qs:], kT[hrows, tk * W:(tk + 1) * W],
                            qT[hrows, t * QW + qs:(t + 1) * QW],
                            start=True, stop=True)
                        e = es_pool.tile([128, QW], F32R, tag="es")
                        nc.scalar.activation(out=e[:, qs:], in_=ps_s[:, qs:],
                                             func=Exp, scale=SCALE)
                        es.append(e)
                    for h in range(HPC):
                        nc.tensor.matmul(ps_o[h][:, qs:], vn[h][:, tk, :],
                                         es[h][:, qs:],
                                         start=(tk == 0), stop=(tk == n_tk - 1))
                # normalize: rows 0..63 / row 64 (broadcast then recip keeps
                # the DVE lane-parallel instead of a 1-partition reciprocal)
                for h in range(HPC):
                    hrows = slice(h * HD, (h + 1) * HD)
                    r_sb = work.tile([1, QW], F32, tag="rsb")
                    nc.vector.tensor_copy(r_sb[:], ps_o[h][HD:, :])
                    bc = work.tile([HD, QW], F32, tag="bc")
                    nc.gpsimd.partition_broadcast(bc[:], r_sb[:])
                    rec = work.tile([HD, QW], F32, tag="rec")
                    nc.vector.reciprocal(out=rec[:], in_=bc[:])
                    nc.vector.tensor_mul(outT[hrows, cols], ps_o[h][:HD, :], rec[:])

                # partial output projection for this query chunk
                for oc in range(ND):
                    ps_f = pf.tile([128, QW], F32, tag="f")
                    nc.tensor.matmul(ps_f[:], wo[:, oc, :], outT[:, cols],
                                     start=True, stop=True)
                    cf = work.tile([128, QW], F32, tag="cf")
                    nc.vector.tensor_copy(cf[:], ps_f[:])
                    nc.sync.dma_start(out=rs_in[t, oc * 128:(oc + 1) * 128, :],
                                      in_=cf[:])

                # chunked ReduceScatter overlaps remaining attention compute
                nc.gpsimd.collective_compute(
                    "ReduceScatter", mybir.AluOpType.add,
                    replica_groups=[list(range(N_CORES))],
                    ins=[rs_in[t].opt()], outs=[rs_out[t].opt()])
                nc.sync.dma_start(out=y_ap[:, cols], in_=rs_out[t])


def kernel(x, Wq, Wk, Wv, Wo):
    if "nc" not in _compiled:
        _compiled["nc"] = _build()
    nc = _compiled["nc"]

    xT = np.ascontiguousarray(x.reshape(T, D).T.astype(np.float32))
    in_maps = []
    for c in range(N_CORES):
        hs = slice(c * HS, (c + 1) * HS)
        in_maps.append({
            "xT": xT,
            "wq": np.ascontiguousarray(Wq[:, hs].astype(np.float32)),
            "wk": np.ascontiguousarray(Wk[:, hs].astype(np.float32)),
            "wv": np.ascontiguousarray(Wv[:, hs].astype(np.float32)),
            "wo": np.ascontiguousarray(Wo[hs, :].astype(np.float32)),
        })
    res = run_bass_kernel_spmd(nc, in_maps, list(range(N_CORES)))
    finalT = np.concatenate([res.results[c]["y"] for c in range(N_CORES)], axis=0)
    return np.ascontiguousarray(finalT.T).reshape(B, T, D)


# revision 14
# speedup vs baseline: 1.1914x; 1.1054x over previous
"""LocalWindowAttention (block-causal) Trainium2 kernel, 8 NeuronCores.

Sharding: tensor-parallel over heads. Core c owns head-columns
[c*128, (c+1)*128) of the D=1024 hidden dim (2 heads x head_dim 64):
  - computes Q/K/V projections for its head slice (transposed layout),
  - block-causal attention for its 2 heads,
  - partial output projection with its 128 rows of Wo,
  - chunked ReduceScatter(add) sums partials; core c keeps rows
    [c*128,(c+1)*128) of final^T [1024, 2048]. Host reassembles.

All big matmuls run in float32r (fp32 with ~13-bit mantissa rounding on
the PE read path): 1 cycle/row for free dim >= 256 -- 4x faster than
plain fp32, ~32x more precise than bf16.

Attention runs in S^T layout (keys on partitions, queries free):
S^T tile = K_chunk @ Q^T. No max-subtraction needed (scores bounded),
and the softmax denominator comes free from a ones-column appended to
the V operand of the attn@V matmul (output row 64 = sum_k exp(s)).
The two heads are interleaved so the exp (ACT engine) of one head
hides behind the other head's matmuls, keeping the PE dense and the
HAM clock un-throttled. Query chunks are processed in descending
visibility order so each chunk's partial output projection and its
ReduceScatter slice overlap the remaining attention compute.
"""

import numpy as np

import concourse.bacc as bacc
import concourse.tile as tile
from concourse import mybir
from concourse.bass_utils import run_bass_kernel_spmd
from concourse.masks import make_identity

B, T, D = 1, 2048, 1024
H, HD, W = 16, 64, 128
N_CORES = 8
HS = D // N_CORES        # 128 head-columns per core (2 heads)
HPC = H // N_CORES       # heads per core
QW = 512                 # query-chunk width (free dim of S^T tiles)
NQ = T // QW             # 4 query chunks
NK = T // W              # 16 key chunks of 128
ND = D // 128            # 8 contraction chunks over D
SCALE = HD ** -0.5

F32 = mybir.dt.float32
F32R = mybir.dt.float32r
Exp = mybir.ActivationFunctionType.Exp

_compiled = {}


def _build():
    nc = bacc.Bacc("TRN2", target_bir_lowering=False, debug=False,
                   num_devices=N_CORES)
    xT_ap = nc.dram_tensor("xT", [D, T], F32R, kind="ExternalInput").ap()
    wq_ap = nc.dram_tensor("wq", [D, HS], F32R, kind="ExternalInput").ap()
    wk_ap = nc.dram_tensor("wk", [D, HS], F32R, kind="ExternalInput").ap()
    wv_ap = nc.dram_tensor("wv", [D, HS], F32R, kind="ExternalInput").ap()
    wo_ap = nc.dram_tensor("wo", [D, HS], F32R, kind="ExternalInput").ap()
    y_ap = nc.dram_tensor("y", [HS, T], F32, kind="ExternalOutput").ap()

    with tile.TileContext(nc) as tc:
        _body(tc, xT_ap, wq_ap, wk_ap, wv_ap, wo_ap, y_ap)
    nc.compile()
    return nc


def _body(tc, xT_ap, wq_ap, wk_ap, wv_ap, wo_ap, y_ap):
    nc = tc.nc
    from contextlib import ExitStack
    with ExitStack() as ctx:
        singles = ctx.enter_context(tc.tile_pool(name="singles", bufs=1))
        work = ctx.enter_context(tc.tile_pool(name="work", bufs=4))
        es_pool = ctx.enter_context(tc.tile_pool(name="es_pool", bufs=6))
        dram = ctx.enter_context(tc.tile_pool(name="dram", bufs=1, space="DRAM"))

        # ---- load inputs (weights first: small, unblock first matmuls) ----
        wq = singles.tile([128, ND, HS], F32R, tag="wq")
        wk = singles.tile([128, ND, HS], F32R, tag="wk")
        wv = singles.tile([128, ND, HS], F32R, tag="wv")
        nc.sync.dma_start(out=wq[:], in_=wq_ap.rearrange("(c p) m -> p c m", p=128))
        nc.sync.dma_start(out=wk[:], in_=wk_ap.rearrange("(c p) m -> p c m", p=128))
        nc.sync.dma_start(out=wv[:], in_=wv_ap.rearrange("(c p) m -> p c m", p=128))
        wo = singles.tile([128, ND, HS], F32R, tag="wo")
        nc.sync.dma_start(out=wo[:], in_=wo_ap.rearrange("(c p) m -> p c m", p=128))
        xts = []
        for d in range(ND):
            xt = singles.tile([128, T], F32R, tag=f"x{d}", name=f"xt{d}")
            eng = nc.sync if d % 2 == 0 else nc.gpsimd
            eng.dma_start(out=xt[:], in_=xT_ap[d * 128:(d + 1) * 128, :])
            xts.append(xt)

        ident_f32 = singles.tile([128, 128], F32, tag="ident_f32")
        make_identity(nc, ident_f32)
        ident = singles.tile([128, 128], F32R, tag="ident")
        nc.vector.tensor_copy(ident[:], ident_f32[:])

        qT = singles.tile([128, T], F32R, tag="qT")
        kT = singles.tile([128, T], F32R, tag="kT")
        vT = singles.tile([128, T], F32R, tag="vT")
        # V in natural layout per head: [key 128, NK chunks, HD + ones col]
        vn = [singles.tile([128, NK, HD + 1], F32R, tag=f"vn{h}", name=f"vn{h}")
              for h in range(HPC)]
        outT = singles.tile([128, T], F32R, tag="outT")

        # ---- Q/K/V projections (transposed layout) ------------------------
        with tc.tile_pool(name="pp", bufs=2, space="PSUM") as pp:
            for t in range(NQ):
                ps_q = pp.tile([128, QW], F32, tag="q")
                ps_k = pp.tile([128, QW], F32, tag="k")
                ps_v = pp.tile([128, QW], F32, tag="v")
                cols = slice(t * QW, (t + 1) * QW)
                for d in range(ND):
                    f = (d == 0)
                    l = (d == ND - 1)
                    nc.tensor.matmul(ps_q[:], wq[:, d, :], xts[d][:, cols], start=f, stop=l)
                    nc.tensor.matmul(ps_k[:], wk[:, d, :], xts[d][:, cols], start=f, stop=l)
                    nc.tensor.matmul(ps_v[:], wv[:, d, :], xts[d][:, cols], start=f, stop=l)
                nc.vector.tensor_copy(qT[:, cols], ps_q[:])
                nc.vector.tensor_copy(kT[:, cols], ps_k[:])
                nc.vector.tensor_copy(vT[:, cols], ps_v[:])

        # ---- transpose V to natural layout, append ones column ------------
        ones = singles.tile([128, 1], F32, tag="ones")
        nc.vector.memset(ones[:], 1.0)
        for h in range(HPC):
            nc.vector.tensor_copy(vn[h][:, :, HD:],
                                  ones[:].unsqueeze(1).to_broadcast([128, NK, 1]))
        with tc.tile_pool(name="pt", bufs=3, space="PSUM") as pt:
            for tk in range(NK):
                ps_t = pt.tile([128, 128], F32R, tag="t")
                nc.tensor.transpose(
                    ps_t[:], vT[:, tk * W:(tk + 1) * W], ident[:])
                for h in range(HPC):
                    nc.vector.tensor_copy(vn[h][:, tk, :HD],
                                          ps_t[:, h * HD:(h + 1) * HD])

        # ---- attention + output projection, query chunks descending -------
        ag_in = [dram.tile([HS, QW], F32R, name=f"ag_in{t}") for t in range(NQ)]
        ag_out = [dram.tile([N_CORES, HS, QW], F32R, addr_space="Shared",
                            name=f"ag_out{t}") for t in range(NQ)]

        with tc.tile_pool(name="pa", bufs=2, space="PSUM") as pa, \
             tc.tile_pool(name="po", bufs=2, space="PSUM") as po, \
             tc.tile_pool(name="pf", bufs=2, space="PSUM") as pf, \
             tc.tile_pool(name="gt_pool", bufs=4) as gt_pool:
            for t in range(NQ - 1, -1, -1):
                cols = slice(t * QW, (t + 1) * QW)
                n_tk = 4 * t + 4
                ps_o = [po.tile([HD + 1, QW], F32, tag=f"o{h}", name=f"ps_o{h}")
                        for h in range(HPC)]

                def s_exp(tk):
                    qs = max(0, (tk - 4 * t) * W)  # masked cols before qs
                    es = []
                    for h in range(HPC):
                        hrows = slice(h * HD, (h + 1) * HD)
                        ps_s = pa.tile([128, QW], F32, tag="s", name="ps_s")
                        nc.tensor.matmul(
                            ps_s[:, qs:], kT[hrows, tk * W:(tk + 1) * W],
                            qT[hrows, t * QW + qs:(t + 1) * QW],
                            start=True, stop=True)
                        e = es_pool.tile([128, QW], F32R, tag="es", name="es")
                        nc.scalar.activation(out=e[:, qs:], in_=ps_s[:, qs:],
                                             func=Exp, scale=SCALE)
                        es.append(e)
                    return es

                def av(tk, es):
                    qs = max(0, (tk - 4 * t) * W)
                    for h in range(HPC):
                        nc.tensor.matmul(ps_o[h][:, qs:], vn[h][:, tk, :],
                                         es[h][:, qs:],
                                         start=(tk == 0), stop=(tk == n_tk - 1))

                # software pipeline: scores/exp run one key-chunk ahead of
                # the attn@V accumulation so the PE never waits on the ACT
                prev = s_exp(0)
                for tk in range(1, n_tk):
                    cur = s_exp(tk)
                    av(tk - 1, prev)
                    prev = cur
                av(n_tk - 1, prev)

                # normalize: rows 0..63 / row 64. One packed reciprocal
                # (cost is driven by the free size, so batching the heads
                # halves it), then a gpsimd partition-broadcast per head.
                for h in range(HPC):
                    hrows = slice(h * HD, (h + 1) * HD)
                    r_sb = work.tile([1, QW], F32, tag="rsb")
                    nc.vector.tensor_copy(r_sb[:], ps_o[h][HD:, :])
                    # 1/r = exp(-ln r) on the scalar engine: ~5x cheaper
                    # than the DVE reciprocal (which costs ~6.5ns/elem)
                    lnr = work.tile([1, QW], F32, tag="lnr")
                    nc.scalar.activation(out=lnr[:], in_=r_sb[:],
                                         func=mybir.ActivationFunctionType.Ln)
                    rec1 = work.tile([1, QW], F32, tag="rec1")
                    nc.scalar.activation(out=rec1[:], in_=lnr[:], func=Exp,
                                         scale=-1.0)
                    bc = work.tile([HD, QW], F32, tag="bc")
                    nc.gpsimd.partition_broadcast(bc[:], rec1[:])
                    nc.vector.tensor_mul(outT[hrows, cols], ps_o[h][:HD, :], bc[:])

                # AllGather this chunk of outT across the 8 cores (overlaps
                # the remaining attention compute), then apply the full Wo
                # to the gathered activations for our 128 output columns
                nc.sync.dma_start(out=ag_in[t][:], in_=outT[:, cols])
                nc.gpsimd.collective_compute(
                    "AllGather", mybir.AluOpType.bypass,
                    replica_groups=[list(range(N_CORES))],
                    ins=[ag_in[t].opt()], outs=[ag_out[t].opt()])
                ps_y = pf.tile([128, QW], F32, tag="y", name="ps_y")
                for c in range(N_CORES):
                    g = gt_pool.tile([128, QW], F32R, tag="g", name="g")
                    nc.sync.dma_start(out=g[:], in_=ag_out[t][c])
                    nc.tensor.matmul(ps_y[:], wo[:, c, :], g[:],
                                     start=(c == 0), stop=(c == N_CORES - 1))
                cy = work.tile([128, QW], F32, tag="cy")
                nc.vector.tensor_copy(cy[:], ps_y[:])
                nc.sync.dma_start(out=y_ap[:, cols], in_=cy[:])


def kernel(x, Wq, Wk, Wv, Wo):
    if "nc" not in _compiled:
        _compiled["nc"] = _build()
    nc = _compiled["nc"]

    xT = np.ascontiguousarray(x.reshape(T, D).T.astype(np.float32))
    in_maps = []
    for c in range(N_CORES):
        hs = slice(c * HS, (c + 1) * HS)
        in_maps.append({
            "xT": xT,
            "wq": np.ascontiguousarray(Wq[:, hs].astype(np.float32)),
            "wk": np.ascontiguousarray(Wk[:, hs].astype(np.float32)),
            "wv": np.ascontiguousarray(Wv[:, hs].astype(np.float32)),
            "wo": np.ascontiguousarray(Wo[:, hs].astype(np.float32)),
        })
    res = run_bass_kernel_spmd(nc, in_maps, list(range(N_CORES)))
    finalT = np.concatenate([res.results[c]["y"] for c in range(N_CORES)], axis=0)
    return np.ascontiguousarray(finalT.T).reshape(B, T, D)


# revision 15
# speedup vs baseline: 1.3078x; 1.0977x over previous
"""LocalWindowAttention (block-causal) Trainium2 kernel, 8 NeuronCores.

Sharding: tensor-parallel over heads. Core c owns head-columns
[c*128, (c+1)*128) of the D=1024 hidden dim (2 heads x head_dim 64):
  - computes Q/K/V projections for its head slice (transposed layout),
  - block-causal attention for its 2 heads,
  - partial output projection with its 128 rows of Wo,
  - chunked ReduceScatter(add) sums partials; core c keeps rows
    [c*128,(c+1)*128) of final^T [1024, 2048]. Host reassembles.

All big matmuls run in float32r (fp32 with ~13-bit mantissa rounding on
the PE read path): 1 cycle/row for free dim >= 256 -- 4x faster than
plain fp32, ~32x more precise than bf16.

Attention runs in S^T layout (keys on partitions, queries free):
S^T tile = K_chunk @ Q^T. No max-subtraction needed (scores bounded),
and the softmax denominator comes free from a ones-column appended to
the V operand of the attn@V matmul (output row 64 = sum_k exp(s)).
The two heads are interleaved so the exp (ACT engine) of one head
hides behind the other head's matmuls, keeping the PE dense and the
HAM clock un-throttled. Query chunks are processed in descending
visibility order so each chunk's partial output projection and its
ReduceScatter slice overlap the remaining attention compute.
"""

import numpy as np

import concourse.bacc as bacc
import concourse.tile as tile
from concourse import mybir
from concourse.bass_utils import run_bass_kernel_spmd
from concourse.masks import make_identity

B, T, D = 1, 2048, 1024
H, HD, W = 16, 64, 128
N_CORES = 8
HS = D // N_CORES        # 128 head-columns per core (2 heads)
HPC = H // N_CORES       # heads per core
QW = 512                 # query-chunk width (free dim of S^T tiles)
NQ = T // QW             # 4 query chunks
NK = T // W              # 16 key chunks of 128
ND = D // 128            # 8 contraction chunks over D
SCALE = HD ** -0.5

F32 = mybir.dt.float32
F32R = mybir.dt.float32r
Exp = mybir.ActivationFunctionType.Exp

_compiled = {}


def _build():
    nc = bacc.Bacc("TRN2", target_bir_lowering=False, debug=False,
                   num_devices=N_CORES)
    xT_ap = nc.dram_tensor("xT", [D, T], F32R, kind="ExternalInput").ap()
    wq_ap = nc.dram_tensor("wq", [D, HS], F32R, kind="ExternalInput").ap()
    wk_ap = nc.dram_tensor("wk", [D, HS], F32R, kind="ExternalInput").ap()
    wv_ap = nc.dram_tensor("wv", [D, HS], F32R, kind="ExternalInput").ap()
    wo_ap = nc.dram_tensor("wo", [D, HS], F32R, kind="ExternalInput").ap()
    y_ap = nc.dram_tensor("y", [HS, T], F32, kind="ExternalOutput").ap()

    with tile.TileContext(nc) as tc:
        _body(tc, xT_ap, wq_ap, wk_ap, wv_ap, wo_ap, y_ap)
    nc.compile()
    return nc


def _body(tc, xT_ap, wq_ap, wk_ap, wv_ap, wo_ap, y_ap):
    nc = tc.nc
    from contextlib import ExitStack
    with ExitStack() as ctx:
        singles = ctx.enter_context(tc.tile_pool(name="singles", bufs=1))
        work = ctx.enter_context(tc.tile_pool(name="work", bufs=4))
        es_pool = ctx.enter_context(tc.tile_pool(name="es_pool", bufs=8))
        dram = ctx.enter_context(tc.tile_pool(name="dram", bufs=1, space="DRAM"))

        # ---- load inputs (weights first: small, unblock first matmuls) ----
        wq = singles.tile([128, ND, HS], F32R, tag="wq")
        wk = singles.tile([128, ND, HS], F32R, tag="wk")
        wv = singles.tile([128, ND, HS], F32R, tag="wv")
        nc.sync.dma_start(out=wq[:], in_=wq_ap.rearrange("(c p) m -> p c m", p=128))
        nc.sync.dma_start(out=wk[:], in_=wk_ap.rearrange("(c p) m -> p c m", p=128))
        nc.sync.dma_start(out=wv[:], in_=wv_ap.rearrange("(c p) m -> p c m", p=128))
        xts = []
        for d in range(ND):
            xt = singles.tile([128, T], F32R, tag=f"x{d}", name=f"xt{d}")
            nc.sync.dma_start(out=xt[:], in_=xT_ap[d * 128:(d + 1) * 128, :])
            xts.append(xt)
        wo = singles.tile([128, ND, HS], F32R, tag="wo")
        nc.sync.dma_start(out=wo[:], in_=wo_ap.rearrange("(c p) m -> p c m", p=128))

        ident_f32 = singles.tile([128, 128], F32, tag="ident_f32")
        make_identity(nc, ident_f32)
        ident = singles.tile([128, 128], F32R, tag="ident")
        nc.vector.tensor_copy(ident[:], ident_f32[:])

        qT = singles.tile([128, T], F32R, tag="qT")
        kT = singles.tile([128, T], F32R, tag="kT")
        vT = singles.tile([128, T], F32R, tag="vT")
        # V in natural layout per head: [key 128, NK chunks, HD + ones col]
        vn = [singles.tile([128, NK, HD + 1], F32R, tag=f"vn{h}", name=f"vn{h}")
              for h in range(HPC)]
        outT = singles.tile([128, T], F32R, tag="outT")

        # ---- Q/K/V projections (transposed layout) ------------------------
        with tc.tile_pool(name="pp", bufs=2, space="PSUM") as pp:
            for t in range(NQ):
                ps_q = pp.tile([128, QW], F32, tag="q")
                ps_k = pp.tile([128, QW], F32, tag="k")
                ps_v = pp.tile([128, QW], F32, tag="v")
                cols = slice(t * QW, (t + 1) * QW)
                for d in range(ND):
                    f = (d == 0)
                    l = (d == ND - 1)
                    nc.tensor.matmul(ps_q[:], wq[:, d, :], xts[d][:, cols], start=f, stop=l)
                    nc.tensor.matmul(ps_k[:], wk[:, d, :], xts[d][:, cols], start=f, stop=l)
                    nc.tensor.matmul(ps_v[:], wv[:, d, :], xts[d][:, cols], start=f, stop=l)
                nc.vector.tensor_copy(qT[:, cols], ps_q[:])
                nc.vector.tensor_copy(kT[:, cols], ps_k[:])
                nc.vector.tensor_copy(vT[:, cols], ps_v[:])

        # ---- transpose V to natural layout, append ones column ------------
        ones = singles.tile([128, 1], F32, tag="ones")
        nc.vector.memset(ones[:], 1.0)
        for h in range(HPC):
            nc.vector.tensor_copy(vn[h][:, :, HD:],
                                  ones[:].unsqueeze(1).to_broadcast([128, NK, 1]))
        with tc.tile_pool(name="pt", bufs=3, space="PSUM") as pt:
            for tk in range(NK):
                ps_t = pt.tile([128, 128], F32R, tag="t")
                nc.tensor.transpose(
                    ps_t[:], vT[:, tk * W:(tk + 1) * W], ident[:])
                for h in range(HPC):
                    nc.vector.tensor_copy(vn[h][:, tk, :HD],
                                          ps_t[:, h * HD:(h + 1) * HD])

        # ---- attention + output projection, query chunks descending -------
        ag_in = [dram.tile([HS, QW], F32R, name=f"ag_in{t}") for t in range(NQ)]
        ag_out = [dram.tile([N_CORES, HS, QW], F32R, addr_space="Shared",
                            name=f"ag_out{t}") for t in range(NQ)]

        with tc.tile_pool(name="pa", bufs=3, space="PSUM") as pa, \
             tc.tile_pool(name="po", bufs=2, space="PSUM") as po, \
             tc.tile_pool(name="pf", bufs=1, space="PSUM") as pf, \
             tc.tile_pool(name="gt_pool", bufs=4) as gt_pool:
            for t in range(NQ - 1, -1, -1):
                cols = slice(t * QW, (t + 1) * QW)
                n_tk = 4 * t + 4
                ps_o = [po.tile([HD + 1, QW], F32, tag=f"o{h}", name=f"ps_o{h}")
                        for h in range(HPC)]

                def s_exp(tk):
                    qs = max(0, (tk - 4 * t) * W)  # masked cols before qs
                    es = []
                    for h in range(HPC):
                        hrows = slice(h * HD, (h + 1) * HD)
                        ps_s = pa.tile([128, QW], F32, tag="s", name="ps_s")
                        nc.tensor.matmul(
                            ps_s[:, qs:], kT[hrows, tk * W:(tk + 1) * W],
                            qT[hrows, t * QW + qs:(t + 1) * QW],
                            start=True, stop=True)
                        e = es_pool.tile([128, QW], F32R, tag="es", name="es")
                        nc.scalar.activation(out=e[:, qs:], in_=ps_s[:, qs:],
                                             func=Exp, scale=SCALE)
                        es.append(e)
                    return es

                def av(tk, es):
                    qs = max(0, (tk - 4 * t) * W)
                    for h in range(HPC):
                        nc.tensor.matmul(ps_o[h][:, qs:], vn[h][:, tk, :],
                                         es[h][:, qs:],
                                         start=(tk == 0), stop=(tk == n_tk - 1))

                # software pipeline: scores/exp run one key-chunk ahead of
                # the attn@V accumulation so the PE never waits on the ACT
                prev = s_exp(0)
                for tk in range(1, n_tk):
                    cur = s_exp(tk)
                    av(tk - 1, prev)
                    prev = cur
                av(n_tk - 1, prev)

                # normalize: rows 0..63 / row 64. One packed reciprocal
                # (cost is driven by the free size, so batching the heads
                # halves it), then a gpsimd partition-broadcast per head.
                for h in range(HPC):
                    hrows = slice(h * HD, (h + 1) * HD)
                    r_sb = work.tile([1, QW], F32, tag="rsb")
                    nc.vector.tensor_copy(r_sb[:], ps_o[h][HD:, :])
                    rec1 = work.tile([1, QW], F32, tag="rec1")
                    nc.vector.reciprocal(out=rec1[:], in_=r_sb[:])
                    bc = work.tile([HD, QW], F32, tag="bc")
                    nc.gpsimd.partition_broadcast(bc[:], rec1[:])
                    nc.vector.tensor_mul(outT[hrows, cols], ps_o[h][:HD, :], bc[:])

                # AllGather this chunk of outT across the 8 cores (overlaps
                # the remaining attention compute), then apply the full Wo
                # to the gathered activations for our 128 output columns
                nc.sync.dma_start(out=ag_in[t][:], in_=outT[:, cols])
                nc.gpsimd.collective_compute(
                    "AllGather", mybir.AluOpType.bypass,
                    replica_groups=[list(range(N_CORES))],
                    ins=[ag_in[t].opt()], outs=[ag_out[t].opt()])
                ps_y = pf.tile([128, QW], F32, tag="y", name="ps_y")
                for c in range(N_CORES):
                    g = gt_pool.tile([128, QW], F32R, tag="g", name="g")
                    nc.sync.dma_start(out=g[:], in_=ag_out[t][c])
                    nc.tensor.matmul(ps_y[:], wo[:, c, :], g[:],
                                     start=(c == 0), stop=(c == N_CORES - 1))
                cy = work.tile([128, QW], F32, tag="cy")
                nc.vector.tensor_copy(cy[:], ps_y[:])
                nc.sync.dma_start(out=y_ap[:, cols], in_=cy[:])


def kernel(x, Wq, Wk, Wv, Wo):
    if "nc" not in _compiled:
        _compiled["nc"] = _build()
    nc = _compiled["nc"]

    xT = np.ascontiguousarray(x.reshape(T, D).T.astype(np.float32))
    in_maps = []
    for c in range(N_CORES):
        hs = slice(c * HS, (c + 1) * HS)
        in_maps.append({
            "xT": xT,
            "wq": np.ascontiguousarray(Wq[:, hs].astype(np.float32)),
            "wk": np.ascontiguousarray(Wk[:, hs].astype(np.float32)),
            "wv": np.ascontiguousarray(Wv[:, hs].astype(np.float32)),
            "wo": np.ascontiguousarray(Wo[:, hs].astype(np.float32)),
        })
    res = run_bass_kernel_spmd(nc, in_maps, list(range(N_CORES)))
    finalT = np.concatenate([res.results[c]["y"] for c in range(N_CORES)], axis=0)
    return np.ascontiguousarray(finalT.T).reshape(B, T, D)


# revision 16
# speedup vs baseline: 1.3807x; 1.0558x over previous
"""LocalWindowAttention (block-causal) Trainium2 kernel, 8 NeuronCores.

Sharding: tensor-parallel over heads. Core c owns head-columns
[c*128, (c+1)*128) of the D=1024 hidden dim (2 heads x head_dim 64):
  - computes Q/K/V projections for its head slice (transposed layout),
  - block-causal attention for its 2 heads,
  - partial output projection with its 128 rows of Wo,
  - chunked ReduceScatter(add) sums partials; core c keeps rows
    [c*128,(c+1)*128) of final^T [1024, 2048]. Host reassembles.

All big matmuls run in float32r (fp32 with ~13-bit mantissa rounding on
the PE read path): 1 cycle/row for free dim >= 256 -- 4x faster than
plain fp32, ~32x more precise than bf16.

Attention runs in S^T layout (keys on partitions, queries free):
S^T tile = K_chunk @ Q^T. No max-subtraction needed (scores bounded),
and the softmax denominator comes free from a ones-column appended to
the V operand of the attn@V matmul (output row 64 = sum_k exp(s)).
The two heads are interleaved so the exp (ACT engine) of one head
hides behind the other head's matmuls, keeping the PE dense and the
HAM clock un-throttled. Query chunks are processed in descending
visibility order so each chunk's partial output projection and its
ReduceScatter slice overlap the remaining attention compute.
"""

import numpy as np

import concourse.bacc as bacc
import concourse.tile as tile
from concourse import mybir
from concourse.bass_utils import run_bass_kernel_spmd
from concourse.masks import make_identity

B, T, D = 1, 2048, 1024
H, HD, W = 16, 64, 128
N_CORES = 8
HS = D // N_CORES        # 128 head-columns per core (2 heads)
HPC = H // N_CORES       # heads per core
QW = 512                 # query-chunk width (free dim of S^T tiles)
NQ = T // QW             # 4 query chunks
NK = T // W              # 16 key chunks of 128
ND = D // 128            # 8 contraction chunks over D
SCALE = HD ** -0.5

F32 = mybir.dt.float32
F32R = mybir.dt.float32r
Exp = mybir.ActivationFunctionType.Exp

_compiled = {}


def _build():
    nc = bacc.Bacc("TRN2", target_bir_lowering=False, debug=False,
                   num_devices=N_CORES)
    xT_ap = nc.dram_tensor("xT", [D, T], F32R, kind="ExternalInput").ap()
    wq_ap = nc.dram_tensor("wq", [D, HS], F32R, kind="ExternalInput").ap()
    wk_ap = nc.dram_tensor("wk", [D, HS], F32R, kind="ExternalInput").ap()
    wv_ap = nc.dram_tensor("wv", [D, HS], F32R, kind="ExternalInput").ap()
    wo_ap = nc.dram_tensor("wo", [D, HS], F32R, kind="ExternalInput").ap()
    y_ap = nc.dram_tensor("y", [HS, T], F32, kind="ExternalOutput").ap()

    with tile.TileContext(nc) as tc:
        _body(tc, xT_ap, wq_ap, wk_ap, wv_ap, wo_ap, y_ap)
    nc.compile()
    return nc


def _body(tc, xT_ap, wq_ap, wk_ap, wv_ap, wo_ap, y_ap):
    nc = tc.nc
    from contextlib import ExitStack
    with ExitStack() as ctx:
        singles = ctx.enter_context(tc.tile_pool(name="singles", bufs=1))
        work = ctx.enter_context(tc.tile_pool(name="work", bufs=4))
        es_pool = ctx.enter_context(tc.tile_pool(name="es_pool", bufs=8))
        dram = ctx.enter_context(tc.tile_pool(name="dram", bufs=1, space="DRAM"))

        # ---- load inputs (weights first: small, unblock first matmuls) ----
        wq = singles.tile([128, ND, HS], F32R, tag="wq")
        wk = singles.tile([128, ND, HS], F32R, tag="wk")
        wv = singles.tile([128, ND, HS], F32R, tag="wv")
        nc.sync.dma_start(out=wq[:], in_=wq_ap.rearrange("(c p) m -> p c m", p=128))
        nc.sync.dma_start(out=wk[:], in_=wk_ap.rearrange("(c p) m -> p c m", p=128))
        nc.sync.dma_start(out=wv[:], in_=wv_ap.rearrange("(c p) m -> p c m", p=128))
        xts = []
        for d in range(ND):
            xt = singles.tile([128, T], F32R, tag=f"x{d}", name=f"xt{d}")
            nc.sync.dma_start(out=xt[:], in_=xT_ap[d * 128:(d + 1) * 128, :])
            xts.append(xt)
        wo = singles.tile([128, ND, HS], F32R, tag="wo")
        nc.sync.dma_start(out=wo[:], in_=wo_ap.rearrange("(c p) m -> p c m", p=128))

        ident_f32 = singles.tile([128, 128], F32, tag="ident_f32")
        make_identity(nc, ident_f32)
        ident = singles.tile([128, 128], F32R, tag="ident")
        nc.vector.tensor_copy(ident[:], ident_f32[:])

        qT = singles.tile([128, T], F32R, tag="qT")
        kT = singles.tile([128, T], F32R, tag="kT")
        vT = singles.tile([128, T], F32R, tag="vT")
        # V in natural layout per head: [key 128, NK chunks, HD + ones col]
        vn = [singles.tile([128, NK, HD + 1], F32R, tag=f"vn{h}", name=f"vn{h}")
              for h in range(HPC)]
        outT = singles.tile([128, T], F32R, tag="outT")

        # ---- Q/K/V projections (transposed layout) ------------------------
        with tc.tile_pool(name="pp", bufs=2, space="PSUM") as pp:
            for t in range(NQ):
                ps_q = pp.tile([128, QW], F32, tag="q")
                ps_k = pp.tile([128, QW], F32, tag="k")
                ps_v = pp.tile([128, QW], F32, tag="v")
                cols = slice(t * QW, (t + 1) * QW)
                for d in range(ND):
                    f = (d == 0)
                    l = (d == ND - 1)
                    nc.tensor.matmul(ps_q[:], wq[:, d, :], xts[d][:, cols], start=f, stop=l)
                    nc.tensor.matmul(ps_k[:], wk[:, d, :], xts[d][:, cols], start=f, stop=l)
                    nc.tensor.matmul(ps_v[:], wv[:, d, :], xts[d][:, cols], start=f, stop=l)
                nc.vector.tensor_copy(qT[:, cols], ps_q[:])
                nc.vector.tensor_copy(kT[:, cols], ps_k[:])
                nc.vector.tensor_copy(vT[:, cols], ps_v[:])

        # ---- transpose V to natural layout, append ones column ------------
        ones = singles.tile([128, 1], F32, tag="ones")
        nc.vector.memset(ones[:], 1.0)
        for h in range(HPC):
            nc.vector.tensor_copy(vn[h][:, :, HD:],
                                  ones[:].unsqueeze(1).to_broadcast([128, NK, 1]))
        with tc.tile_pool(name="pt", bufs=3, space="PSUM") as pt:
            for tk in range(NK):
                ps_t = pt.tile([128, 128], F32R, tag="t")
                nc.tensor.transpose(
                    ps_t[:], vT[:, tk * W:(tk + 1) * W], ident[:])
                for h in range(HPC):
                    nc.vector.tensor_copy(vn[h][:, tk, :HD],
                                          ps_t[:, h * HD:(h + 1) * HD])

        # ---- attention + output projection, query chunks descending -------
        ag_in = [dram.tile([HS, QW], F32R, name=f"ag_in{t}") for t in range(NQ)]
        ag_out = [dram.tile([N_CORES, HS, QW], F32R, addr_space="Shared",
                            name=f"ag_out{t}") for t in range(NQ)]

        with tc.tile_pool(name="pa", bufs=3, space="PSUM") as pa, \
             tc.tile_pool(name="po", bufs=2, space="PSUM") as po:
            for t in range(NQ - 1, -1, -1):
                cols = slice(t * QW, (t + 1) * QW)
                n_tk = 4 * t + 4
                ps_o = [po.tile([HD + 1, QW], F32, tag=f"o{h}", name=f"ps_o{h}")
                        for h in range(HPC)]

                def s_exp(tk):
                    qs = max(0, (tk - 4 * t) * W)  # masked cols before qs
                    es = []
                    for h in range(HPC):
                        hrows = slice(h * HD, (h + 1) * HD)
                        ps_s = pa.tile([128, QW], F32, tag="s", name="ps_s")
                        nc.tensor.matmul(
                            ps_s[:, qs:], kT[hrows, tk * W:(tk + 1) * W],
                            qT[hrows, t * QW + qs:(t + 1) * QW],
                            start=True, stop=True)
                        e = es_pool.tile([128, QW], F32R, tag="es", name="es")
                        nc.scalar.activation(out=e[:, qs:], in_=ps_s[:, qs:],
                                             func=Exp, scale=SCALE)
                        es.append(e)
                    return es

                def av(tk, es):
                    qs = max(0, (tk - 4 * t) * W)
                    for h in range(HPC):
                        nc.tensor.matmul(ps_o[h][:, qs:], vn[h][:, tk, :],
                                         es[h][:, qs:],
                                         start=(tk == 0), stop=(tk == n_tk - 1))

                # software pipeline: scores/exp run one key-chunk ahead of
                # the attn@V accumulation so the PE never waits on the ACT
                prev = s_exp(0)
                for tk in range(1, n_tk):
                    cur = s_exp(tk)
                    av(tk - 1, prev)
                    prev = cur
                av(n_tk - 1, prev)

                # normalize: rows 0..63 / row 64
                for h in range(HPC):
                    hrows = slice(h * HD, (h + 1) * HD)
                    r_sb = work.tile([1, QW], F32, tag="rsb")
                    nc.vector.tensor_copy(r_sb[:], ps_o[h][HD:, :])
                    rec1 = work.tile([1, QW], F32, tag="rec1")
                    nc.vector.reciprocal(out=rec1[:], in_=r_sb[:])
                    bc = work.tile([HD, QW], F32, tag="bc")
                    nc.gpsimd.partition_broadcast(bc[:], rec1[:])
                    nc.vector.tensor_mul(outT[hrows, cols], ps_o[h][:HD, :], bc[:])

                # kick off this chunk's AllGather; its consumers are emitted
                # after ALL attention so the in-order PE never stalls on it
                nc.sync.dma_start(out=ag_in[t][:], in_=outT[:, cols])
                nc.gpsimd.collective_compute(
                    "AllGather", mybir.AluOpType.bypass,
                    replica_groups=[list(range(N_CORES))],
                    ins=[ag_in[t].opt()], outs=[ag_out[t].opt()])

        # apply the full Wo to the gathered activations: for our 128 output
        # columns, final^T[c-slice, cols] = sum_c Wo[c-block, slice]^T @ outT_c
        with tc.tile_pool(name="pf", bufs=2, space="PSUM") as pf, \
             tc.tile_pool(name="gt_pool", bufs=4) as gt_pool:
            for t in range(NQ - 1, -1, -1):
                cols = slice(t * QW, (t + 1) * QW)
                ps_y = pf.tile([128, QW], F32, tag="y", name="ps_y")
                for c in range(N_CORES):
                    g = gt_pool.tile([128, QW], F32R, tag="g", name="g")
                    nc.sync.dma_start(out=g[:], in_=ag_out[t][c])
                    nc.tensor.matmul(ps_y[:], wo[:, c, :], g[:],
                                     start=(c == 0), stop=(c == N_CORES - 1))
                cy = work.tile([128, QW], F32, tag="cy")
                nc.vector.tensor_copy(cy[:], ps_y[:])
                nc.sync.dma_start(out=y_ap[:, cols], in_=cy[:])


def kernel(x, Wq, Wk, Wv, Wo):
    if "nc" not in _compiled:
        _compiled["nc"] = _build()
    nc = _compiled["nc"]

    xT = np.ascontiguousarray(x.reshape(T, D).T.astype(np.float32))
    in_maps = []
    for c in range(N_CORES):
        hs = slice(c * HS, (c + 1) * HS)
        in_maps.append({
            "xT": xT,
            "wq": np.ascontiguousarray(Wq[:, hs].astype(np.float32)),
            "wk": np.ascontiguousarray(Wk[:, hs].astype(np.float32)),
            "wv": np.ascontiguousarray(Wv[:, hs].astype(np.float32)),
            "wo": np.ascontiguousarray(Wo[:, hs].astype(np.float32)),
        })
    res = run_bass_kernel_spmd(nc, in_maps, list(range(N_CORES)))
    finalT = np.concatenate([res.results[c]["y"] for c in range(N_CORES)], axis=0)
    return np.ascontiguousarray(finalT.T).reshape(B, T, D)
